# revision 1
# baseline (speedup 1.0000x reference)
"""Trainium2 Bass kernel for a 4-layer causal-attention LM.

Model: V=32000, D=1024, H=16 heads, L=4 layers, B=2, S=1024.
  x = emb[tokens] + pos_enc
  per layer: q,k,v = x@W; causal softmax attention; out-proj; residual; LN
  logits = x @ out_w

Sharding over 8 NeuronCores (per sharding hint):
  DP=2 over batch  x  Megatron TP=4 over heads.
  Core c: batch g=c//4, rank r=c%4 owns heads [4r, 4r+4) and vocab cols
  [8000r, 8000(r+1)). Attention/QKV column-parallel, out-proj row-parallel
  with a per-512-column-chunk AllReduce (pipelined). Final vocab projection
  is column-parallel; the host concatenates shards (no collective).

Layout: activations kept feature-major ("xT": [d partitions, seq free]) so
every matmul contracts over partitions with zero transposes (only the
embedding entry needs PE transposes). Matmuls run as float32r (FP22,
full PE rate); the exp/V attention operands run bf16. Softmax uses
transposed scores [sk, sq]; per-query sums come free from a ones column
appended to V in the A@V matmul; normalization folds into the ctx
eviction. LayerNorm stats (feature-axis) via ones-vector matmuls.
"""

import numpy as np

V, D, H, L = 32000, 1024, 16, 4
B, S = 2, 1024
HD = D // H            # 64
P = 128
NG = 4                 # TP degree (cores per batch group)
HL = H // NG           # 4 heads per core
HCOLS = HL * HD        # 256 projection cols per core
VS = V // NG           # 8000 vocab shard
DT = D // P            # 8 d-tiles
SQC = 512              # seq chunk for AR pipelining
NSQC = S // SQC        # 2
NT = S // P            # 8 seq tiles
VC = 500               # vocab tile (8000 = 16*500)
NVC = VS // VC         # 16
SCALE = 1.0 / float(np.sqrt(HD))
EPS = 1e-5
NEG = -1.0e9
RG = [[0, 1, 2, 3], [4, 5, 6, 7]]

_COMPILED = None  # cache (nc) across calls


def _pos_encoding():
    pos = np.arange(S, dtype=np.float32)[:, None]
    div = np.exp(np.arange(0, D, 2, dtype=np.float32) * (-np.log(10000.0) / D))
    ang = pos * div
    pe = np.stack([np.sin(ang), np.cos(ang)], axis=-1).reshape(S, D)
    return pe.astype(np.float32)


def _build():
    import concourse.bass as bass
    import concourse.tile as tile
    from concourse import bacc, mybir

    f32 = mybir.dt.float32
    f32r = mybir.dt.float32r
    bf16 = mybir.dt.bfloat16
    i32 = mybir.dt.int32
    AF = mybir.ActivationFunctionType

    nc = bacc.Bacc("TRN2", target_bir_lowering=False, debug=False, num_devices=8)

    tok = nc.dram_tensor("tok", [S, 1], i32, kind="ExternalInput").ap()
    ident_d = nc.dram_tensor("ident", [P, P], f32, kind="ExternalInput").ap()
    ones_d = nc.dram_tensor("onesc", [P, 1], f32r, kind="ExternalInput").ap()
    masks_d = nc.dram_tensor("masks", [P, 4 * SQC], f32, kind="ExternalInput").ap()
    emb = nc.dram_tensor("emb", [V, D], f32, kind="ExternalInput").ap()
    peT = nc.dram_tensor("peT", [D, S], f32, kind="ExternalInput").ap()
    qw = nc.dram_tensor("qw", [L, D, HCOLS], bf16, kind="ExternalInput").ap()
    kw = nc.dram_tensor("kw", [L, D, HCOLS], bf16, kind="ExternalInput").ap()
    vw = nc.dram_tensor("vw", [L, D, HCOLS], bf16, kind="ExternalInput").ap()
    ow = nc.dram_tensor("ow", [L, D, D], bf16, kind="ExternalInput").ap()
    outw = nc.dram_tensor("outw", [D, VS], bf16, kind="ExternalInput").ap()
    out = nc.dram_tensor("out", [S, VS], f32, kind="ExternalOutput").ap()

    with tile.TileContext(nc) as tc:
        with (
            tc.tile_pool(name="const", bufs=1) as constp,
            tc.tile_pool(name="xp", bufs=1) as xp,
            tc.tile_pool(name="psum", bufs=2, space="PSUM") as psp,
        ):
            # ---- constants (host-provided: walrus chokes on affine_select) ----
            ident = constp.tile([P, P], f32)
            nc.sync.dma_start(out=ident[:], in_=ident_d[:])
            ones = constp.tile([P, 1], f32r)
            nc.sync.dma_start(out=ones[:], in_=ones_d[:])
            epsb = constp.tile([1, 1], f32)
            nc.vector.memset(epsb[:], EPS)
            # additive causal masks for the 4 diagonal sk-tiles of each sq
            # chunk: mask[trel][i, j] = 0 if j >= 128*trel + i else NEG
            masks = constp.tile([P, 4, SQC], f32)
            nc.sync.dma_start(
                out=masks[:], in_=masks_d.rearrange("p (t s) -> p t s", t=4)
            )

            # persistent activations, feature-major: x[d, s], d = a*128 + p
            xT = xp.tile([P, DT, S], f32r)
            xTb0 = xp.tile([P, DT, SQC], bf16, name="xTb0")
            xTb1 = xp.tile([P, DT, SQC], bf16, name="xTb1")
            xTbs = (xTb0, xTb1)

            # ---- embedding: gather rows, transpose to feature-major, +pe ----
            with tc.tile_pool(name="embp", bufs=2) as embp:
                tokt = embp.tile([P, NT], i32, bufs=1)
                nc.sync.dma_start(
                    out=tokt[:], in_=tok.rearrange("(t p) o -> p (t o)", p=P)
                )
                for st in range(NT):
                    xrow = embp.tile([P, D], f32, tag="xrow")
                    nc.gpsimd.indirect_dma_start(
                        out=xrow[:],
                        out_offset=None,
                        in_=emb[:],
                        in_offset=bass.IndirectOffsetOnAxis(
                            ap=tokt[:, st : st + 1], axis=0
                        ),
                    )
                    pesb = embp.tile([P, DT, P], f32, tag="pesb")
                    nc.sync.dma_start(
                        out=pesb[:],
                        in_=peT[:, st * P : (st + 1) * P].rearrange(
                            "(a p) s -> p a s", p=P
                        ),
                    )
                    for dc in range(DT):
                        tps = psp.tile([P, P], f32, tag="mm", name=f"tps_{st}_{dc}")
                        nc.tensor.transpose(
                            tps[:], xrow[:, dc * P : (dc + 1) * P], ident[:]
                        )
                        nc.vector.tensor_add(
                            xT[:, dc, st * P : (st + 1) * P],
                            tps[:],
                            pesb[:, dc, :],
                        )
                        nc.scalar.copy(
                            xTbs[st // (NT // 2)][
                                :, dc, (st % (NT // 2)) * P : (st % (NT // 2) + 1) * P
                            ],
                            xT[:, dc, st * P : (st + 1) * P],
                        )

            # ---- transformer layers ----
            with (
                tc.tile_pool(name="wp", bufs=3) as wp,
                tc.tile_pool(name="owp", bufs=1) as owp,
                tc.tile_pool(name="apl", bufs=1) as apool,
                tc.tile_pool(name="expp", bufs=4) as expp,
                tc.tile_pool(name="lnp", bufs=1) as lnp,
                tc.tile_pool(name="dcp", bufs=2) as dcp,
                tc.tile_pool(name="small", bufs=1) as smallp,
                tc.tile_pool(name="dram", bufs=2, space="DRAM") as dramp,
            ):
                # Per-(layer, chunk) stage emitters. Engine instruction
                # streams execute in emission order, so the pipelined order
                # below is what hides each chunk's AllReduce behind the other
                # chunk's attention / the next layer's QKV.

                def load_weights(l):
                    w = {}
                    for nm, src in (("qw", qw), ("kw", kw), ("vw", vw)):
                        t = wp.tile([P, DT, HCOLS], bf16, tag="w", name=f"{nm}{l}")
                        nc.sync.dma_start(
                            out=t[:], in_=src[l].rearrange("(a p) m -> p a m", p=P)
                        )
                        w[nm] = t
                    t = owp.tile([P, DT, D], bf16, tag="ow", name=f"ow{l}")
                    nc.sync.dma_start(
                        out=t[:], in_=ow[l].rearrange("(a p) m -> p a m", p=P)
                    )
                    w["ow"] = t
                    return w

                def qkv(l, c, w, stl):
                    # q,k feature-major [headcol, s] (head h: partitions
                    # 64*(h%2).., chunk h//2); v seq-major bf16 with a ones
                    # column at 64 for free softmax sums.
                    if c == 0:
                        stl["qT"] = apool.tile(
                            [P, 2, S], bf16, tag="qT", name=f"qT{l}"
                        )
                        stl["kT"] = apool.tile(
                            [P, 2, S], bf16, tag="kT", name=f"kT{l}"
                        )
                        stl["vS"] = apool.tile(
                            [P, NT, HL, 66], bf16, tag="vS", name=f"vS{l}"
                        )
                        stl["ctx"] = apool.tile(
                            [P, 2, S], bf16, tag="ctx", name=f"ctx{l}"
                        )
                    qT, kT, vS = stl["qT"], stl["kT"], stl["vS"]
                    xTb = xTbs[c]
                    for dst, wsb in ((qT, w["qw"]), (kT, w["kw"])):
                        for hp in range(2):
                            ps = psp.tile([P, SQC], f32, tag="mm")
                            for kt in range(DT):
                                nc.tensor.matmul(
                                    ps[:],
                                    lhsT=wsb[:, kt, hp * P : (hp + 1) * P],
                                    rhs=xTb[:, kt, :],
                                    start=(kt == 0),
                                    stop=(kt == DT - 1),
                                )
                            nc.scalar.copy(dst[:, hp, c * SQC : (c + 1) * SQC], ps[:])
                    for st in range(4 * c, 4 * c + 4):
                        lt = st - 4 * c
                        nc.vector.memset(vS[:, st, :, 64:65], 1.0)
                        ps = psp.tile([P, HCOLS], f32, tag="mm")
                        for kt in range(DT):
                            nc.tensor.matmul(
                                ps[:],
                                lhsT=xTb[:, kt, lt * P : (lt + 1) * P],
                                rhs=w["vw"][:, kt, :],
                                start=(kt == 0),
                                stop=(kt == DT - 1),
                            )
                        nc.vector.tensor_copy(
                            vS[:, st, :, 0:64],
                            ps[:].rearrange("p (h e) -> p h e", h=HL),
                        )

                def att_gen(l, c, stl):
                    # transposed scores [sk, sq]; ctx feature-major
                    qT, kT, vS, ctx = stl["qT"], stl["kT"], stl["vS"], stl["ctx"]
                    nt_vis = 4 * c + 4
                    for hpair in range(HL // 2):
                        hs = (2 * hpair, 2 * hpair + 1)
                        avs = {}
                        for h in hs:
                            avs[h] = psp.tile(
                                [P, SQC], f32, tag="av", name=f"av{h}"
                            )
                        # phase-split in blocks of 4 sk-tiles: all scores
                        # (PE dense, exp chases on ACT), then all A@V
                        for tb in range(0, nt_vis, 4):
                            exs = {}
                            for t in range(tb, tb + 4):
                                for h in hs:
                                    hp, hr = divmod(h, 2)
                                    p0 = 64 * hr
                                    sc = psp.tile(
                                        [P, SQC], f32, tag="sc", bufs=4
                                    )
                                    nc.tensor.matmul(
                                        sc[:],
                                        lhsT=kT[p0 : p0 + 64, hp, t * P : (t + 1) * P],
                                        rhs=qT[p0 : p0 + 64, hp, c * SQC : (c + 1) * SQC],
                                        start=True,
                                        stop=True,
                                    )
                                    trel = t - 4 * c
                                    if trel >= 0:
                                        nc.vector.tensor_add(
                                            sc[:], sc[:], masks[:, trel, :]
                                        )
                                    ex = expp.tile(
                                        [P, SQC], bf16, tag="ex", bufs=10
                                    )
                                    nc.scalar.activation(
                                        ex[:], sc[:], AF.Exp, scale=SCALE
                                    )
                                    exs[(h, t)] = ex
                            yield
                            for t in range(tb, tb + 4):
                                for h in hs:
                                    nc.tensor.matmul(
                                        avs[h][0:65, :],
                                        lhsT=vS[:, t, h, 0:65],
                                        rhs=exs[(h, t)][:],
                                        start=(t == 0),
                                        stop=(t == nt_vis - 1),
                                    )
                            yield
                        for h in hs:
                            hp, hr = divmod(h, 2)
                            p0 = 64 * hr
                            av = avs[h]
                            ssum = smallp.tile([1, SQC], f32, tag="ssum", bufs=2)
                            nc.scalar.copy(ssum[:], av[64:65, :])
                            inv = smallp.tile([1, SQC], f32, tag="inv", bufs=2)
                            nc.vector.reciprocal_approx_fast(inv[:], ssum[:])
                            invb = smallp.tile([64, SQC], f32, tag="invb", bufs=2)
                            nc.gpsimd.partition_broadcast(invb[:], inv[:])
                            nc.vector.tensor_mul(
                                ctx[p0 : p0 + 64, hp, c * SQC : (c + 1) * SQC],
                                av[0:64, :],
                                invb[:],
                            )

                def ag_ctx(l, c, stl):
                    # all-gather the 4 local heads' ctx across the TP group:
                    # rank r contributes rows [256r, 256(r+1)) = heads 4r..
                    ctx = stl["ctx"]
                    ag_in = dramp.tile(
                        [HCOLS, SQC], bf16, tag="agi", name=f"agi{l}_{c}"
                    )
                    nc.sync.dma_start(
                        out=ag_in.rearrange("(hp p) s -> p hp s", p=P),
                        in_=ctx[:, :, c * SQC : (c + 1) * SQC],
                    )
                    ag_out = dramp.tile(
                        [D, SQC], bf16, tag="ago", name=f"ago{l}_{c}"
                    )
                    nc.gpsimd.collective_compute(
                        "AllGather",
                        mybir.AluOpType.bypass,
                        replica_groups=RG,
                        ins=[ag_in[:].opt()],
                        outs=[ag_out[:].opt()],
                    )
                    stl[f"ag{c}"] = ag_out

                def oprln_gen(l, c, w, stl):
                    # full out-proj on gathered ctx (replicated across the
                    # group), residual fused into the psum eviction, then
                    # feature-axis LN; writes xT chunk c in place.
                    ag_out = stl[f"ag{c}"]
                    ctxF = lnp.tile([P, DT, SQC], bf16, tag="ctxF", bufs=2)
                    nc.sync.dma_start(
                        out=ctxF[:], in_=ag_out.rearrange("(a p) s -> p a s", p=P)
                    )
                    xr = lnp.tile([P, DT, SQC], f32r, tag=f"xr{c}")
                    st0 = psp.tile([1, SQC], f32, tag="mm")
                    st1 = psp.tile([1, SQC], f32, tag="mm")
                    for dc in range(DT):
                        ps = psp.tile([P, SQC], f32, tag="sc", bufs=4)
                        for kt in range(DT):
                            nc.tensor.matmul(
                                ps[:],
                                lhsT=w["ow"][:, kt, dc * P : (dc + 1) * P],
                                rhs=ctxF[:, kt, :],
                                start=(kt == 0),
                                stop=(kt == DT - 1),
                            )
                        nc.vector.tensor_add(
                            xr[:, dc, :], ps[:], xT[:, dc, c * SQC : (c + 1) * SQC]
                        )
                        sqt = dcp.tile([P, SQC], f32r, tag="sqt")
                        nc.vector.tensor_mul(sqt[:], xr[:, dc, :], xr[:, dc, :])
                        nc.tensor.matmul(
                            st0[:],
                            lhsT=ones[:],
                            rhs=xr[:, dc, :],
                            start=(dc == 0),
                            stop=(dc == DT - 1),
                        )
                        nc.tensor.matmul(
                            st1[:],
                            lhsT=ones[:],
                            rhs=sqt[:],
                            start=(dc == 0),
                            stop=(dc == DT - 1),
                        )
                        yield
                    nmean = smallp.tile([1, SQC], f32, tag=f"nmean{c}")
                    nc.scalar.mul(nmean[:], st0[:], -1.0 / D)
                    msq = smallp.tile([1, SQC], f32, tag=f"msq{c}")
                    nc.scalar.activation(msq[:], st0[:], AF.Square, scale=1.0 / D)
                    ex2 = smallp.tile([1, SQC], f32, tag=f"ex2{c}")
                    nc.scalar.mul(ex2[:], st1[:], 1.0 / D)
                    var = smallp.tile([1, SQC], f32, tag=f"var{c}")
                    nc.vector.tensor_sub(var[:], ex2[:], msq[:])
                    std = smallp.tile([1, SQC], f32, tag=f"std{c}")
                    nc.scalar.activation(std[:], var[:], AF.Sqrt, bias=epsb[:])
                    rstd = smallp.tile([1, SQC], f32, tag=f"rstd{c}")
                    nc.vector.reciprocal_approx_fast(rstd[:], std[:])
                    mb = smallp.tile([P, SQC], f32, tag=f"mb{c}")
                    nc.gpsimd.partition_broadcast(mb[:], nmean[:])
                    rb = smallp.tile([P, SQC], f32, tag=f"rb{c}")
                    nc.gpsimd.partition_broadcast(rb[:], rstd[:])
                    for dc in range(DT):
                        nc.vector.tensor_add(xr[:, dc, :], xr[:, dc, :], mb[:])
                        nc.vector.tensor_mul(
                            xT[:, dc, c * SQC : (c + 1) * SQC], xr[:, dc, :], rb[:]
                        )
                        nc.scalar.copy(
                            xTbs[c][:, dc, :], xT[:, dc, c * SQC : (c + 1) * SQC]
                        )

                _DONE = object()

                def zip_emit(*gens, head=0):
                    # interleave emission so out-proj matmuls fill the PE
                    # bubbles of the exp-bound attention pipeline
                    gens = [g for g in gens if g is not None]
                    if head and gens:
                        for _ in range(head):
                            next(gens[0], None)
                    alive = list(gens)
                    while alive:
                        for g in list(alive):
                            if next(g, _DONE) is _DONE:
                                alive.remove(g)

                # pre-warm the collective path (first AllGather pays
                # ~40us extra); overlaps the embedding
                wi = dramp.tile([P, 4], f32, tag="warm")
                nc.sync.dma_start(out=wi[:], in_=masks_d[:, 0:4])
                wo = dramp.tile([4 * P, 4], f32, tag="warm2")
                nc.gpsimd.collective_compute(
                    "AllGather",
                    mybir.AluOpType.bypass,
                    replica_groups=RG,
                    ins=[wi[:].opt()],
                    outs=[wo[:].opt()],
                )

                states = [dict() for _ in range(L)]
                wcur = load_weights(0)
                wprev = None
                for l in range(L):
                    w = wcur
                    stl = states[l]
                    qkv(l, 0, w, stl)
                    zip_emit(att_gen(l, 0, stl))
                    ag_ctx(l, 0, stl)
                    qkv(l, 1, w, stl)
                    zip_emit(att_gen(l, 1, stl))
                    ag_ctx(l, 1, stl)
                    zip_emit(oprln_gen(l, 0, w, stl))
                    zip_emit(oprln_gen(l, 1, w, stl))
                    wprev = w
                    if l + 1 < L:
                        wcur = load_weights(l + 1)
                # final vocab projection (column-parallel, host concat);
                # reuses layer-pool slots (ctxF/sqt) — no new SBUF

                def final_gen(st_lo, st_hi):
                    for vc in range(NVC):
                        wv = lnp.tile([P, DT, VC], bf16, tag="ctxF", bufs=2)
                        nc.sync.dma_start(
                            out=wv[:],
                            in_=outw[:, vc * VC : (vc + 1) * VC].rearrange(
                                "(a p) m -> p a m", p=P
                            ),
                        )
                        for st in range(st_lo, st_hi):
                            half = xTbs[st // (NT // 2)]
                            lst = st % (NT // 2)
                            ps = psp.tile([P, SQC], f32, tag="sc", bufs=4)
                            for kt in range(DT):
                                nc.tensor.matmul(
                                    ps[:, 0:VC],
                                    lhsT=half[:, kt, lst * P : (lst + 1) * P],
                                    rhs=wv[:, kt, :],
                                    start=(kt == 0),
                                    stop=(kt == DT - 1),
                                )
                            ob = dcp.tile([P, VC], f32, tag="sqt")
                            nc.scalar.copy(ob[:], ps[:, 0:VC])
                            nc.sync.dma_start(
                                out=out[
                                    st * P : (st + 1) * P,
                                    vc * VC : (vc + 1) * VC,
                                ],
                                in_=ob[:],
                            )
                        yield

                zip_emit(final_gen(0, NT))
    nc.finalize()
    return nc


def _bf(a):
    import ml_dtypes

    return np.ascontiguousarray(a.astype(ml_dtypes.bfloat16))


def _in_maps(tokens, emb, qw, kw, vw, ow, out_w):
    pe = _pos_encoding()
    peT = np.ascontiguousarray(pe.T)
    ident = np.eye(P, dtype=np.float32)
    j = np.arange(SQC)[None, :]
    i = np.arange(P)[:, None]
    masks = np.concatenate(
        [
            np.where(j >= P * trel + i, 0.0, NEG).astype(np.float32)
            for trel in range(4)
        ],
        axis=1,
    )
    masks = np.ascontiguousarray(masks)
    maps = []
    for c in range(8):
        g, r = divmod(c, NG)
        hc0 = r * HCOLS
        maps.append(
            {
                "tok": np.ascontiguousarray(
                    tokens[g].reshape(S, 1).astype(np.int32)
                ),
                "ident": ident,
                "onesc": np.ones((P, 1), dtype=np.float32),
                "masks": masks,
                "emb": emb,
                "peT": peT,
                "qw": _bf(qw[:, :, hc0 : hc0 + HCOLS]),
                "kw": _bf(kw[:, :, hc0 : hc0 + HCOLS]),
                "vw": _bf(vw[:, :, hc0 : hc0 + HCOLS]),
                "ow": _bf(ow),
                "outw": _bf(out_w[:, r * VS : (r + 1) * VS]),
            }
        )
    return maps


def run(inputs, trace=False):
    """Build+compile (cached), run on 8 cores, return (full_output, results)."""
    global _COMPILED
    from concourse.bass_utils import run_bass_kernel_spmd

    if _COMPILED is None:
        _COMPILED = _build()
    nc = _COMPILED

    tokens = np.asarray(inputs["tokens"])
    maps = _in_maps(
        np.asarray(tokens),
        np.ascontiguousarray(np.asarray(inputs["emb"], dtype=np.float32)),
        np.asarray(inputs["qw"], dtype=np.float32),
        np.asarray(inputs["kw"], dtype=np.float32),
        np.asarray(inputs["vw"], dtype=np.float32),
        np.asarray(inputs["ow"], dtype=np.float32),
        np.ascontiguousarray(np.asarray(inputs["out_w"], dtype=np.float32)),
    )
    res = run_bass_kernel_spmd(nc, maps, core_ids=list(range(8)), trace=trace)
    full = np.empty((B, S, V), dtype=np.float32)
    for c in range(8):
        g, r = divmod(c, NG)
        full[g, :, r * VS : (r + 1) * VS] = res.results[c]["out"]
    return full, res


def kernel(**inputs):
    full, _ = run(inputs)
    return full

